# revision 1
# baseline (speedup 1.0000x reference)
"""TRN2 Bass kernel for nn_Denoise: 2x100-iteration FISTA row denoise.

Scheme (per core, data-parallel over batch):
  Layout: transposed per image -> tiles [128 elem-partitions, 8192] where
  column = chunk*2048 + img*512 + row  (4 chunks of 128 row-elements).
  Iteration: v = k1*z - k2*(A @ f32r(z)) + s*y   (A = D^T D, integer entries,
  exact in float32r; computed on PE accumulating over ACT-preloaded alpha*y),
  m = min(v, y) ;  x = relu(m) ;  z' = (1+b)*relu(m) - b*relu(m_prev).
  The fp32 k1*z path keeps full precision; quantization error A*(z - f32r(z))
  is spectrally annihilated where the QP resolvent would amplify it.
"""
import numpy as np

LAM = 10.0
N_ITERS = 100
N = 512
NIMG_PER_CORE = 4
NCORES = 8
FD = NIMG_PER_CORE * N          # 2048 columns per chunk
NCHUNK = 4                      # 512 = 4 * 128 row-elements
TOT = NCHUNK * FD               # 8192
G = 4                           # groups (psum pipelining)
GFD = FD // G                   # 512 columns per chunk per group

_cache = {}


def _f32r(v):
    b = np.ascontiguousarray(v, np.float32).view(np.uint32)
    r = b + 0x7FF + ((b >> 12) & 1)
    r &= np.uint32(0xFFFFF000)
    return r.view(np.float32)


def _host_consts():
    step = np.float32(1.0 / (2.0 * (1.0 + 16.0 * LAM)))
    k1 = float(1.0 - 2.0 * float(step))
    k2 = float(2.0 * LAM * float(step))
    s = float(2.0 * float(step))
    alpha = -s / k2  # = -1/LAM
    # beta sequence in fp32 exactly like the reference
    t = np.float32(1.0)
    b0 = np.zeros(N_ITERS, np.float32)
    b1 = np.zeros(N_ITERS, np.float32)
    for k in range(N_ITERS):
        tn = np.float32(0.5) * (np.float32(1.0) + np.sqrt(np.float32(1.0) + np.float32(4.0) * t * t))
        beta = (t - np.float32(1.0)) / tn
        b0[k] = beta
        b1[k] = np.float32(1.0) + beta
        t = tn
    # A = D^T D (integer entries)
    D = np.zeros((N - 2, N), np.float64)
    idx = np.arange(N - 2)
    D[idx, idx] = 1.0
    D[idx, idx + 1] = -2.0
    D[idx, idx + 2] = 1.0
    A = D.T @ D
    # distinct lhsT blocks: lhsT[k, m] = A[jbase + k, ibase + m]
    A00 = A[0:128, 0:128]
    AII = A[128:256, 128:256]
    A33 = A[384:512, 384:512]
    BU = A[128:256, 0:128]      # cj = c+1 feeding c
    BL = A[0:128, 128:256]      # cj = c-1 feeding c
    wts = np.concatenate([A00, AII, A33, BU, BL, np.eye(128)], axis=1).astype(np.float32)  # [128, 768]
    betas = np.concatenate([b1, b0])[None, :].repeat(128, axis=0).astype(np.float32)  # [128, 200]
    return step, k1, k2, s, alpha, wts, betas


def _register_ops():
    from concourse import dve_ops
    from concourse.dve_spec import Spec, Src0, Src1, C0, C1, lower
    from concourse.dve_spec import _has_src1 as has_src1
    from concourse.dve_spec import relu as drelu
    from concourse.dve_uop import DveOpSpec

    def register_op(name, spec):
        for o in dve_ops.OPS:
            if o.name == name:
                return o
        op = dve_ops.DveOp(name, spec, False, uops_sha={})
        dve_ops.OPS.append(op)
        row = dve_ops._CUSTOM_DVE_ROW_BASE + len(dve_ops.OPS) - 1
        assert row < 0x20
        dve_ops._SUB_OPCODE_FOR_NAME[name] = row
        dve_ops.CUSTOM_DVE_SPECS[name] = spec
        sha = {}
        for ver in ("v3", "v4"):
            sp = DveOpSpec(name=name, opcode=row, uops=lower(spec, ver=ver),
                           rd1_en=has_src1(spec))
            sha[ver] = sp.sha(ver)
        op2 = dve_ops.DveOp(name, spec, False, uops_sha=sha)
        dve_ops.OPS[-1] = op2
        return op2

    fista_v = register_op(
        "FISTA_V",
        Spec(body=(Src0 - Src1 * C0) * C1,
             reference=lambda in0, in1, s0, s1, imm2: (in0 - in1 * s0) * s1))
    from concourse.dve_spec import minn
    fista_m = register_op(
        "FISTA_M",
        Spec(body=minn(Src0 + Src1 * C0, Src1),
             reference=lambda in0, in1, s0, s1, imm2: np.minimum(in0 + in1 * s0, in1)))
    fista_z = register_op(
        "FISTA_Z",
        Spec(body=drelu(Src0) * C0 - drelu(Src1) * C1,
             reference=lambda in0, in1, s0, s1, imm2:
             np.maximum(in0, 0) * s0 - np.maximum(in1, 0) * s1))
    return fista_v, fista_m, fista_z


N_ITERS_RUN = [N_ITERS]
MM_MODE = ["full"]
DUMP_PS = [False]
RDT = ["f32r"]   # rounding dtype for z/weights: f32r or bf16
NHALF = [1]
SKIP = [set()]
LOOP_MODE = ["dynamic"]
PASSES = [2]

def _build(trace=False):
    key = (N_ITERS_RUN[0], LOOP_MODE[0], PASSES[0], MM_MODE[0], DUMP_PS[0], tuple(sorted(SKIP[0])), RDT[0], NHALF[0])
    if key in _cache:
        return _cache[key]
    import concourse.bacc as bacc
    import concourse.tile as tile
    from concourse import mybir
    import concourse.bass as bass

    FISTA_V, FISTA_M, FISTA_Z = _register_ops()
    step, k1, k2, s, alpha, wts_np, betas_np = _host_consts()
    f32 = mybir.dt.float32
    f32r = mybir.dt.float32r if RDT[0] == "f32r" else mybir.dt.bfloat16

    nc = bacc.Bacc("TRN2", target_bir_lowering=False, debug=False)
    DATA = nc.dram_tensor("data", [NIMG_PER_CORE, N, N], f32, kind="ExternalInput")
    WTS = nc.dram_tensor("wts", [128, 6 * 128], f32, kind="ExternalInput")
    BET = nc.dram_tensor("betas", [128, 2 * N_ITERS], f32, kind="ExternalInput")
    OUT = nc.dram_tensor("out", [NIMG_PER_CORE, N, N], f32, kind="ExternalOutput")

    with tile.TileContext(nc) as tc:
        with (
            tc.tile_pool(name="state", bufs=1) as pool,
            tc.tile_pool(name="psum", bufs=2, space="PSUM") as psp,
            tc.tile_pool(name="scratch", bufs=1) as pool2,
        ):
            y_t = pool.tile([128, TOT], f32, tag="y")
            z_t = pool.tile([128, TOT], f32, tag="z")
            zr_a = pool.tile([128, TOT], f32r, tag="zra")
            zr_b = pool.tile([128, TOT], f32r, tag="zrb")
            m_a = pool.tile([128, TOT], f32, tag="ma")
            m_b = pool.tile([128, TOT], f32, tag="mb")
            w_t = pool.tile([128, 6 * 128], f32, tag="w")
            wr_t = pool.tile([128, 5 * 128], f32r, tag="wr")
            stg_pool = pool
            bet_t = pool.tile([128, 2 * N_ITERS], f32, tag="bet")

            # ---- load weights/betas
            nc.sync.dma_start(w_t[:], WTS[:])
            nc.sync.dma_start(bet_t[:], BET[:])
            nc.vector.tensor_copy(wr_t[:], w_t[:, :5 * 128])
            ident = w_t[:, 5 * 128:6 * 128]
            WBLK = {  # (cj - c) -> per-c lhsT slice index into wr_t
                ("d", 0): 0, ("d", 1): 1, ("d", 2): 1, ("d", 3): 2,
            }

            def wslice(idx):
                return wr_t[:, idx * 128:(idx + 1) * 128]

            # ---- load input + transpose on PE:
            # y[p, c*FD + i*N + (128t+r')] = data[i, 128t+r', 128c+p]
            for i in range(NIMG_PER_CORE):
                for t_ in range(4):
                    sbase = ((i * 4 + t_) % 4) * N
                    nc.sync.dma_start(m_b[:, sbase:sbase + N],
                                      DATA[i, 128 * t_:128 * (t_ + 1), :])
                    psT = psp.tile([128, NCHUNK * GFD], f32, tag="ps")
                    for c in range(NCHUNK):
                        nc.tensor.transpose(psT[:, 128 * c:128 * (c + 1)],
                                            m_b[:, sbase + 128 * c:sbase + 128 * (c + 1)], ident)
                    dstv = y_t[:].rearrange("p (c n) -> p c n", c=NCHUNK)[
                        :, :, i * N + 128 * t_: i * N + 128 * (t_ + 1)]
                    nc.vector.tensor_copy(
                        dstv, psT[:, :N].rearrange("p (c n) -> p c n", c=NCHUNK))

            def init_state():
                nc.vector.tensor_copy(z_t[:], y_t[:])
                nc.gpsimd.tensor_copy(zr_a[:], y_t[:])
                nc.scalar.copy(m_a[:], y_t[:])

            def g3(t, g):
                """[128, 4, GFD] view of group g of a [128, TOT] tile."""
                return t[:].rearrange("p (c n) -> p c n", c=NCHUNK)[:, :, g * GFD:(g + 1) * GFD]

            HW = FD // NHALF[0]   # interleaved independent column groups

            def iteration(m_in, m_out, zr_in, zr_out, h, i_b1, i_b0):
                hb = h * HW
                for c in range(NCHUNK):
                    cs = slice(c * FD + hb, c * FD + hb + HW)
                    ps = psp.tile([128, HW], mybir.dt.float32, tag="ps")
                    mlist = [(WBLK[("d", c)], c)]
                    if c + 1 < NCHUNK:
                        mlist.append((3, c + 1))
                    if c - 1 >= 0:
                        mlist.append((4, c - 1))
                    if "mm" in SKIP[0]:
                        mlist = []
                        nc.scalar.activation(ps[:], y_t[:, cs], mybir.ActivationFunctionType.Copy)
                    for p in range(HW // 512):
                        for bi, (widx, cj) in enumerate(mlist):
                            nc.tensor.matmul(
                                ps[:, p * 512:(p + 1) * 512], wslice(widx),
                                zr_in[:, cj * FD + hb + p * 512: cj * FD + hb + (p + 1) * 512],
                                start=(bi == 0),
                                stop=(bi == len(mlist) - 1),
                                skip_group_check=True,
                            )
                    if "dve" not in SKIP[0]:
                        # v = (psum - z*(k1/k2)) * (-k2)
                        vt = pool2.tile([128, HW], mybir.dt.float32, tag="v")
                        nc.vector._custom_dve(FISTA_V, out=vt[:], in0=ps[:],
                                              in1=z_t[:, cs], s0=k1 / k2, s1=-k2)
                        # m = min(v + s*y, y)
                        nc.vector._custom_dve(FISTA_M, out=m_out[:, cs], in0=vt[:],
                                              in1=y_t[:, cs], s0=s)
                        # z' = b1*relu(m) - b0*relu(m_prev)
                        nc.vector._custom_dve(FISTA_Z, out=z_t[:, cs], in0=m_out[:, cs],
                                              in1=m_in[:, cs], s0=i_b1, s1=i_b0)
                    if "gps" not in SKIP[0]:
                        # zr = f32r(z')
                        nc.gpsimd.tensor_copy(zr_out[:, cs], z_t[:, cs])

            def iteration2(m_in, m_out, zr_in, zr_out, i_b1, i_b0):
                for h in range(NHALF[0]):
                    iteration(m_in, m_out, zr_in, zr_out, h, i_b1, i_b0)

            def run_pass():
                nit = N_ITERS_RUN[0]
                if LOOP_MODE[0] == "unrolled":
                    for k in range(0, nit, 2):
                        iteration2(m_a, m_b, zr_a, zr_b, bet_t[:, k:k + 1],
                                  bet_t[:, N_ITERS + k:N_ITERS + k + 1])
                        iteration2(m_b, m_a, zr_b, zr_a, bet_t[:, k + 1:k + 2],
                                  bet_t[:, N_ITERS + k + 1:N_ITERS + k + 2])
                else:
                    def body(i):
                        iteration2(m_a, m_b, zr_a, zr_b,
                                  bet_t[:, bass.ds(i, 1)],
                                  bet_t[:, bass.ds(i + N_ITERS, 1)])
                        iteration2(m_b, m_a, zr_b, zr_a,
                                  bet_t[:, bass.ds(i + 1, 1)],
                                  bet_t[:, bass.ds(i + 1 + N_ITERS, 1)])
                    with tc.For_i(0, nit, 2) as i:
                        body(i)

            init_state()
            run_pass()
            for _extra in range(PASSES[0] - 1):
                # next pass: y <- relu(m_a) (x_100), reinit, run again
                nc.scalar.activation(y_t[:], m_a[:], mybir.ActivationFunctionType.Relu)
                init_state()
                run_pass()
            # final x = relu(m_a) -> z_t as staging
            if not DUMP_PS[0]:
                nc.scalar.activation(z_t[:], m_a[:], mybir.ActivationFunctionType.Relu)

            # store: transpose back on PE then contiguous DMA
            for i in range(NIMG_PER_CORE):
                for t_ in range(4):
                    psT = psp.tile([128, NCHUNK * GFD], f32, tag="ps")
                    for c in range(NCHUNK):
                        nc.tensor.transpose(
                            psT[:, 128 * c:128 * (c + 1)],
                            z_t[:, c * FD + i * N + 128 * t_: c * FD + i * N + 128 * (t_ + 1)],
                            ident)
                    S = m_b[:, ((i * 4 + t_) % 4) * N:(((i * 4 + t_) % 4) + 1) * N]
                    nc.vector.tensor_copy(S, psT[:, :N])
                    nc.sync.dma_start(OUT[i, 128 * t_:128 * (t_ + 1), :], S)

    nc.finalize()
    _cache[key] = nc
    return nc


def kernel(data: np.ndarray) -> np.ndarray:
    from concourse import bass_utils

    data = np.ascontiguousarray(data, np.float32)
    B = data.shape[0]
    nc = _build()
    _, _, _, _, _, wts_np, betas_np = _host_consts()
    in_maps = []
    for c in range(NCORES):
        in_maps.append({
            "data": np.ascontiguousarray(data[c * NIMG_PER_CORE:(c + 1) * NIMG_PER_CORE]),
            "wts": wts_np,
            "betas": betas_np,
        })
    res = bass_utils.run_bass_kernel_spmd(nc, in_maps, core_ids=list(range(NCORES)))
    out = np.concatenate([res.results[c]["out"] for c in range(NCORES)], axis=0)
    return out.reshape(B, N, N, 1)


if __name__ == "__main__":
    rng = np.random.default_rng(0)
    d = rng.random((32, N, N), dtype=np.float32)
    o = kernel(d)
    print("kernel ran, out shape", o.shape, "mean", o.mean())



# revision 2
# speedup vs baseline: 3.1456x; 3.1456x over previous
"""TRN2 Bass kernel for nn_Denoise: 2x100-iteration FISTA row denoise.

Scheme (per core, data-parallel over batch, 4 images/core):
  Layout: transposed per image -> tiles [128 elem-partitions, 8192] where
  column = chunk*2048 + img*512 + row  (4 chunks of 128 row-elements).
  Per iteration, per chunk (2048 cols):
    PE:           P  = A @ f32r(z)   (A = D^T D; 10 psum-accumulated matmuls)
    DVE custom V: u  = (P - z*(k1/k2)) * (-k2)   [= k1 z - k2 A z], in-place in m
    DVE custom M: m  = min(u + s*y, y)           (in-place)
    DVE custom Z: z' = b1*relu(m) - b0*relu(m_prev)
    ACT copy:     zr = f32r(z')      (rounding for the next matmul, off-DVE)
  The fp32 k1*z path keeps full precision; f32r rounding enters only through
  A, which annihilates the smooth modes the FISTA map would amplify.

Perf notes (HW-measured, axon/PJRT):
  - The f32r rounding copy runs on the Activation engine, not gpsimd (the
    Pool engine is a software DSP and TensorTensor is not even legal on it).
  - zr and m are double-buffered so cross-iteration WAR hazards do not
    serialize the 4-chunk pipeline; V/M write in-place to save SBUF.
  - The dynamic loop costs ~300us of device time PER TRIP while the body
    stays under ~2k instructions (engine instruction-queue replay); bigger
    bodies fall off a cliff (~3-4us per instruction per trip: U=50 -> 273
    us/iter, U=100 -> 198 us/iter). Measured per-iteration device time:
    U=2 -> 171 us, U=20 -> 44 us vs ~28 us of raw engine time. UNROLL=20
    (1140-instr body, 5 trips/pass) is the best unroll that divides 100.
    Full static unrolling is no better: this axon runtime pays ~wall
    per-NEFF-instruction per call, so a 12k-instruction NEFF costs ~+1 s
    wall per invocation.
"""
import numpy as np

LAM = 10.0
N_ITERS = 100
N = 512
NIMG_PER_CORE = 4
NCORES = 8
FD = NIMG_PER_CORE * N          # 2048 columns per chunk
NCHUNK = 4                      # 512 = 4 * 128 row-elements
TOT = NCHUNK * FD               # 8192

_cache = {}

# knobs (module-level so experiments can flip them)
N_ITERS_RUN = [N_ITERS]
PASSES = [2]
LOOP_MODE = ["dynamic"]         # "dynamic" | "unrolled"
UNROLL = [20]                     # iterations per loop trip in dynamic mode
ROUND_MODE = ["actcopy"]        # "actcopy" | "dvecopy" | "poolcopy"
VARIANT = ["va"]                # "va": 3 DVE customs, no preload, no pool
                                # "d1": ACT preload + pool min + 2 DVE customs
ZPP = [True]                   # ping-pong z tiles


def _host_consts(nbeta=None):
    if nbeta is None:
        nbeta = max(N_ITERS, N_ITERS_RUN[0])
    step = np.float32(1.0 / (2.0 * (1.0 + 16.0 * LAM)))
    k1 = float(1.0 - 2.0 * float(step))
    k2 = float(2.0 * LAM * float(step))
    s = float(2.0 * float(step))
    alpha = -s / k2  # = -1/LAM
    t = np.float32(1.0)
    b0 = np.zeros(nbeta, np.float32)
    b1 = np.zeros(nbeta, np.float32)
    for k in range(nbeta):
        tn = np.float32(0.5) * (np.float32(1.0) + np.sqrt(np.float32(1.0) + np.float32(4.0) * t * t))
        beta = (t - np.float32(1.0)) / tn
        b0[k] = beta
        b1[k] = np.float32(1.0) + beta
        t = tn
    D = np.zeros((N - 2, N), np.float64)
    idx = np.arange(N - 2)
    D[idx, idx] = 1.0
    D[idx, idx + 1] = -2.0
    D[idx, idx + 2] = 1.0
    A = D.T @ D
    A00 = A[0:128, 0:128]
    AII = A[128:256, 128:256]
    A33 = A[384:512, 384:512]
    BU = A[128:256, 0:128]      # cj = c+1 feeding c
    BL = A[0:128, 128:256]      # cj = c-1 feeding c
    wts = np.concatenate([A00, AII, A33, BU, BL, np.eye(128)], axis=1).astype(np.float32)  # [128, 768]
    betas = np.concatenate([b1, b0])[None, :].repeat(128, axis=0).astype(np.float32)  # [128, 200]
    return step, k1, k2, s, alpha, wts, betas


def _register_ops():
    from concourse import dve_ops
    from concourse.dve_spec import Spec, Src0, Src1, C0, C1, lower
    from concourse.dve_spec import _has_src1 as has_src1
    from concourse.dve_spec import relu as drelu
    from concourse.dve_uop import DveOpSpec

    def register_op(name, spec):
        for o in dve_ops.OPS:
            if o.name == name:
                return o
        op = dve_ops.DveOp(name, spec, False, uops_sha={})
        dve_ops.OPS.append(op)
        row = dve_ops._CUSTOM_DVE_ROW_BASE + len(dve_ops.OPS) - 1
        assert row < 0x20
        dve_ops._SUB_OPCODE_FOR_NAME[name] = row
        dve_ops.CUSTOM_DVE_SPECS[name] = spec
        sha = {}
        for ver in ("v3", "v4"):
            sp = DveOpSpec(name=name, opcode=row, uops=lower(spec, ver=ver),
                           rd1_en=has_src1(spec))
            sha[ver] = sp.sha(ver)
        op2 = dve_ops.DveOp(name, spec, False, uops_sha=sha)
        dve_ops.OPS[-1] = op2
        return op2

    from concourse.dve_spec import minn
    fista_v = register_op(
        "FISTA_V",
        Spec(body=(Src0 - Src1 * C0) * C1,
             reference=lambda in0, in1, s0, s1, imm2: (in0 - in1 * s0) * s1))
    fista_z = register_op(
        "FISTA_Z",
        Spec(body=drelu(Src0) * C0 - drelu(Src1) * C1,
             reference=lambda in0, in1, s0, s1, imm2:
             np.maximum(in0, 0) * s0 - np.maximum(in1, 0) * s1))
    fista_m = register_op(
        "FISTA_M",
        Spec(body=minn(Src0 + Src1 * C0, Src1),
             reference=lambda in0, in1, s0, s1, imm2: np.minimum(in0 + in1 * s0, in1)))
    return fista_v, fista_z, fista_m


def _build(trace=False):
    key = (N_ITERS_RUN[0], LOOP_MODE[0], UNROLL[0], PASSES[0], ROUND_MODE[0],
           VARIANT[0], ZPP[0])
    if key in _cache:
        return _cache[key]
    import concourse.bacc as bacc
    import concourse.tile as tile
    from concourse import mybir
    import concourse.bass as bass

    FISTA_V, FISTA_Z, FISTA_M = _register_ops()
    step, k1, k2, s, alpha, wts_np, betas_np = _host_consts()
    f32 = mybir.dt.float32
    f32r = mybir.dt.float32r

    NBETA = max(N_ITERS, N_ITERS_RUN[0])
    nc = bacc.Bacc("TRN2", target_bir_lowering=False, debug=False)
    DATA = nc.dram_tensor("data", [NIMG_PER_CORE, N, N], f32, kind="ExternalInput")
    WTS = nc.dram_tensor("wts", [128, 6 * 128], f32, kind="ExternalInput")
    BET = nc.dram_tensor("betas", [128, 2 * NBETA], f32, kind="ExternalInput")
    OUT = nc.dram_tensor("out", [NIMG_PER_CORE, N, N], f32, kind="ExternalOutput")

    with tile.TileContext(nc) as tc:
        with (
            tc.tile_pool(name="state", bufs=1) as pool,
            tc.tile_pool(name="psum", bufs=2, space="PSUM") as psp,
            tc.tile_pool(name="scratch", bufs=(1 if ZPP[0] else 2)) as pool2,
        ):
            y_t = pool.tile([128, TOT], f32, tag="y")
            z_a = pool.tile([128, TOT], f32, tag="za")
            z_b = z_a
            zr_a = pool.tile([128, TOT], f32r, tag="zra")
            if ZPP[0]:
                zr_b = pool.tile([128, TOT], f32r, tag="zrb")
            else:
                zr_b = zr_a
            m_a = pool.tile([128, TOT], f32, tag="ma")
            m_b = pool.tile([128, TOT], f32, tag="mb")
            w_t = pool.tile([128, 6 * 128], f32, tag="w")
            wr_t = pool.tile([128, 5 * 128], f32r, tag="wr")
            bet_t = pool.tile([128, 2 * NBETA], f32, tag="bet")

            # ---- load weights/betas
            nc.sync.dma_start(w_t[:], WTS[:])
            nc.sync.dma_start(bet_t[:], BET[:])
            nc.vector.tensor_copy(wr_t[:], w_t[:, :5 * 128])
            ident = w_t[:, 5 * 128:6 * 128]
            WBLK = {0: 0, 1: 1, 2: 1, 3: 2}   # chunk -> diagonal lhsT slice

            def wslice(idx):
                return wr_t[:, idx * 128:(idx + 1) * 128]

            # ---- load input + transpose on PE:
            # y[p, c*FD + i*N + (128t+r')] = data[i, 128t+r', 128c+p]
            for i in range(NIMG_PER_CORE):
                for t_ in range(4):
                    sbase = ((i * 4 + t_) % 4) * N
                    nc.sync.dma_start(m_b[:, sbase:sbase + N],
                                      DATA[i, 128 * t_:128 * (t_ + 1), :])
                    psT = psp.tile([128, 512], f32, tag="ps")
                    for c in range(NCHUNK):
                        nc.tensor.transpose(psT[:, 128 * c:128 * (c + 1)],
                                            m_b[:, sbase + 128 * c:sbase + 128 * (c + 1)], ident)
                    dstv = y_t[:].rearrange("p (c n) -> p c n", c=NCHUNK)[
                        :, :, i * N + 128 * t_: i * N + 128 * (t_ + 1)]
                    nc.vector.tensor_copy(
                        dstv, psT[:, :N].rearrange("p (c n) -> p c n", c=NCHUNK))

            def round_to(dst, src):
                if ROUND_MODE[0] == "actcopy":
                    nc.scalar.activation(dst, src, mybir.ActivationFunctionType.Copy)
                elif ROUND_MODE[0] == "dvecopy":
                    nc.vector.tensor_copy(dst, src)
                else:
                    nc.gpsimd.tensor_copy(dst, src)

            def init_state():
                nc.vector.tensor_copy(z_a[:], y_t[:])
                nc.scalar.copy(m_a[:], y_t[:])
                nc.gpsimd.tensor_copy(zr_a[:], y_t[:])

            def iteration(m_in, m_out, zr_in, zr_out, i_b1, i_b0):
                z_in = z_out = z_a
                for c in range(NCHUNK):
                    cs = slice(c * FD, (c + 1) * FD)
                    ps = psp.tile([128, FD], mybir.dt.float32, tag="ps")
                    preload = VARIANT[0] == "d1"
                    if preload:
                        # PSUM preload: alpha * y
                        nc.scalar.activation(ps[:], y_t[:, cs],
                                             mybir.ActivationFunctionType.Copy,
                                             scale=float(alpha))
                    mlist = [(WBLK[c], c)]
                    if c + 1 < NCHUNK:
                        mlist.append((3, c + 1))
                    if c - 1 >= 0:
                        mlist.append((4, c - 1))
                    rhs_of = lambda cj, lo, hi: zr_in[:, cj * FD + lo: cj * FD + hi]
                    for p in range(FD // 512):
                        for bi, (widx, cj) in enumerate(mlist):
                            nc.tensor.matmul(
                                ps[:, p * 512:(p + 1) * 512], wslice(widx),
                                rhs_of(cj, p * 512, (p + 1) * 512),
                                start=(not preload and bi == 0),
                                stop=(bi == len(mlist) - 1),
                                skip_group_check=True,
                            )
                    if VARIANT[0] == "d1":
                        # u = (P - z*(k1/k2)) * (-k2)  [= k1 z - k2 A z + s y]
                        # written in-place into m_out, then clamped there
                        nc.vector._custom_dve(FISTA_V, out=m_out[:, cs], in0=ps[:],
                                              in1=z_in[:, cs], s0=k1 / k2, s1=-k2)
                        # m = min(u, y)  (Pool, in-place)
                        nc.gpsimd.tensor_tensor(m_out[:, cs], m_out[:, cs], y_t[:, cs],
                                                op=mybir.AluOpType.min)
                    else:
                        # u = (P - z*(k1/k2)) * (-k2)  [= k1 z - k2 A z]
                        # written in-place into m_out, then min'd there
                        nc.vector._custom_dve(FISTA_V, out=m_out[:, cs], in0=ps[:],
                                              in1=z_in[:, cs], s0=k1 / k2, s1=-k2)
                        # m = min(u + s*y, y)  (DVE custom, in-place)
                        nc.vector._custom_dve(FISTA_M, out=m_out[:, cs], in0=m_out[:, cs],
                                              in1=y_t[:, cs], s0=s)
                    # z' = b1*relu(m) - b0*relu(m_prev)
                    nc.vector._custom_dve(FISTA_Z, out=z_out[:, cs], in0=m_out[:, cs],
                                          in1=m_in[:, cs], s0=i_b1, s1=i_b0)
                    # zr = f32r(z') for the next iteration's matmuls
                    round_to(zr_out[:, cs], z_out[:, cs])

            def run_pass():
                nit = N_ITERS_RUN[0]
                if LOOP_MODE[0] == "unrolled":
                    for k in range(0, nit, 2):
                        iteration(m_a, m_b, zr_a, zr_b, bet_t[:, k:k + 1],
                                  bet_t[:, NBETA + k:NBETA + k + 1])
                        iteration(m_b, m_a, zr_b, zr_a, bet_t[:, k + 1:k + 2],
                                  bet_t[:, NBETA + k + 1:NBETA + k + 2])
                else:
                    U = UNROLL[0]
                    assert U % 2 == 0 and nit % U == 0
                    def body(i):
                        for j in range(0, U, 2):
                            iteration(m_a, m_b, zr_a, zr_b,
                                      bet_t[:, bass.ds(i + j, 1)],
                                      bet_t[:, bass.ds(i + j + NBETA, 1)])
                            iteration(m_b, m_a, zr_b, zr_a,
                                      bet_t[:, bass.ds(i + j + 1, 1)],
                                      bet_t[:, bass.ds(i + j + 1 + NBETA, 1)])
                    with tc.For_i(0, nit, U) as i:
                        body(i)

            init_state()
            run_pass()
            for _extra in range(PASSES[0] - 1):
                # next pass: y <- relu(m_a) (x_100), reinit, run again
                nc.scalar.activation(y_t[:], m_a[:], mybir.ActivationFunctionType.Relu)
                init_state()
                run_pass()
            # final x = relu(m_a) -> z_a as staging
            nc.scalar.activation(z_a[:], m_a[:], mybir.ActivationFunctionType.Relu)

            # store: transpose back on PE then contiguous DMA
            for i in range(NIMG_PER_CORE):
                for t_ in range(4):
                    psT = psp.tile([128, 512], f32, tag="ps")
                    for c in range(NCHUNK):
                        nc.tensor.transpose(
                            psT[:, 128 * c:128 * (c + 1)],
                            z_a[:, c * FD + i * N + 128 * t_: c * FD + i * N + 128 * (t_ + 1)],
                            ident)
                    S = m_b[:, ((i * 4 + t_) % 4) * N:(((i * 4 + t_) % 4) + 1) * N]
                    nc.vector.tensor_copy(S, psT[:, :N])
                    nc.sync.dma_start(OUT[i, 128 * t_:128 * (t_ + 1), :], S)

    nc.finalize()
    _cache[key] = nc
    return nc


def kernel(data: np.ndarray) -> np.ndarray:
    from concourse import bass_utils

    data = np.ascontiguousarray(data, np.float32)
    B = data.shape[0]
    nc = _build()
    _, _, _, _, _, wts_np, betas_np = _host_consts()
    in_maps = []
    for c in range(NCORES):
        in_maps.append({
            "data": np.ascontiguousarray(data[c * NIMG_PER_CORE:(c + 1) * NIMG_PER_CORE]),
            "wts": wts_np,
            "betas": betas_np,
        })
    res = bass_utils.run_bass_kernel_spmd(nc, in_maps, core_ids=list(range(NCORES)))
    out = np.concatenate([res.results[c]["out"] for c in range(NCORES)], axis=0)
    return out.reshape(B, N, N, 1)


if __name__ == "__main__":
    rng = np.random.default_rng(0)
    d = rng.random((32, N, N), dtype=np.float32)
    o = kernel(d)
    print("kernel ran, out shape", o.shape, "mean", o.mean())


# revision 3
# speedup vs baseline: 3.8839x; 1.2347x over previous
"""TRN2 Bass kernel for nn_Denoise: 2x100-iteration FISTA row denoise.

Scheme (per core, data-parallel over batch, 4 images/core):
  Layout: transposed per image -> tiles [128 elem-partitions, 8192] where
  column = chunk*2048 + img*512 + row  (4 chunks of 128 row-elements).
  Per iteration, per chunk (2048 cols):
    PE:           P  = A @ f32r(z)   (A = D^T D; 10 psum-accumulated matmuls)
    DVE custom V: u  = (P - z*(k1/k2)) * (-k2)   [= k1 z - k2 A z], in-place in m
    DVE custom M: m  = min(u + s*y, y)           (in-place)
    DVE custom Z: z' = b1*relu(m) - b0*relu(m_prev)
    ACT copy:     zr = f32r(z')      (rounding for the next matmul, off-DVE)
  The fp32 k1*z path keeps full precision; f32r rounding enters only through
  A, which annihilates the smooth modes the FISTA map would amplify.

Perf notes (HW-measured, axon/PJRT):
  - The f32r rounding copy runs on the Activation engine, not gpsimd (the
    Pool engine is a software DSP and TensorTensor is not even legal on it).
  - zr and m are double-buffered so cross-iteration WAR hazards do not
    serialize the 4-chunk pipeline; V/M write in-place to save SBUF.
  - The dynamic loop costs ~300us of device time PER TRIP while the body
    stays under ~2k instructions (engine instruction-queue replay); bigger
    bodies fall off a cliff (~3-4us per instruction per trip: U=50 -> 273
    us/iter, U=100 -> 198 us/iter). Measured per-iteration device time:
    U=2 -> 171 us, U=20 -> 35.7 us vs ~28 us of raw engine time. UNROLL=20
    (1140-instr body, 5 trips/pass) is the best unroll that divides 100.
    Full static unrolling is no better: this axon runtime pays ~wall
    per-NEFF-instruction per call, so a 12k-instruction NEFF costs ~+1 s
    wall per invocation.
"""
import numpy as np

LAM = 10.0
N_ITERS = 100
N = 512
NIMG_PER_CORE = 4
NCORES = 8
FD = NIMG_PER_CORE * N          # 2048 columns per chunk
NCHUNK = 4                      # 512 = 4 * 128 row-elements
TOT = NCHUNK * FD               # 8192

_cache = {}

# knobs (module-level so experiments can flip them)
N_ITERS_RUN = [N_ITERS]
PASSES = [2]
LOOP_MODE = ["dynamic"]         # "dynamic" | "unrolled"
UNROLL = [20]                     # iterations per loop trip in dynamic mode
ROUND_MODE = ["actcopy"]        # "actcopy" | "dvecopy" | "poolcopy"
VARIANT = ["va"]                # "va": 3 DVE customs, no preload, no pool
                                # "d1": ACT preload + pool min + 2 DVE customs
ZPP = [True]                   # ping-pong z tiles


def _host_consts(nbeta=None):
    if nbeta is None:
        nbeta = max(N_ITERS, N_ITERS_RUN[0])
    step = np.float32(1.0 / (2.0 * (1.0 + 16.0 * LAM)))
    k1 = float(1.0 - 2.0 * float(step))
    k2 = float(2.0 * LAM * float(step))
    s = float(2.0 * float(step))
    alpha = -s / k2  # = -1/LAM
    t = np.float32(1.0)
    b0 = np.zeros(nbeta, np.float32)
    b1 = np.zeros(nbeta, np.float32)
    for k in range(nbeta):
        tn = np.float32(0.5) * (np.float32(1.0) + np.sqrt(np.float32(1.0) + np.float32(4.0) * t * t))
        beta = (t - np.float32(1.0)) / tn
        b0[k] = beta
        b1[k] = np.float32(1.0) + beta
        t = tn
    D = np.zeros((N - 2, N), np.float64)
    idx = np.arange(N - 2)
    D[idx, idx] = 1.0
    D[idx, idx + 1] = -2.0
    D[idx, idx + 2] = 1.0
    A = D.T @ D
    A00 = A[0:128, 0:128]
    AII = A[128:256, 128:256]
    A33 = A[384:512, 384:512]
    BU = A[128:256, 0:128]      # cj = c+1 feeding c
    BL = A[0:128, 128:256]      # cj = c-1 feeding c
    wts = np.concatenate([A00, AII, A33, BU, BL, np.eye(128)], axis=1).astype(np.float32)  # [128, 768]
    betas = np.concatenate([b1, b0])[None, :].repeat(128, axis=0).astype(np.float32)  # [128, 200]
    return step, k1, k2, s, alpha, wts, betas


def _register_ops():
    from concourse import dve_ops
    from concourse.dve_spec import Spec, Src0, Src1, C0, C1, lower
    from concourse.dve_spec import _has_src1 as has_src1
    from concourse.dve_spec import relu as drelu
    from concourse.dve_uop import DveOpSpec

    def register_op(name, spec):
        for o in dve_ops.OPS:
            if o.name == name:
                return o
        op = dve_ops.DveOp(name, spec, False, uops_sha={})
        dve_ops.OPS.append(op)
        row = dve_ops._CUSTOM_DVE_ROW_BASE + len(dve_ops.OPS) - 1
        assert row < 0x20
        dve_ops._SUB_OPCODE_FOR_NAME[name] = row
        dve_ops.CUSTOM_DVE_SPECS[name] = spec
        sha = {}
        for ver in ("v3", "v4"):
            sp = DveOpSpec(name=name, opcode=row, uops=lower(spec, ver=ver),
                           rd1_en=has_src1(spec))
            sha[ver] = sp.sha(ver)
        op2 = dve_ops.DveOp(name, spec, False, uops_sha=sha)
        dve_ops.OPS[-1] = op2
        return op2

    from concourse.dve_spec import minn
    fista_v = register_op(
        "FISTA_V",
        Spec(body=(Src0 - Src1 * C0) * C1,
             reference=lambda in0, in1, s0, s1, imm2: (in0 - in1 * s0) * s1))
    fista_z = register_op(
        "FISTA_Z",
        Spec(body=drelu(Src0) * C0 - drelu(Src1) * C1,
             reference=lambda in0, in1, s0, s1, imm2:
             np.maximum(in0, 0) * s0 - np.maximum(in1, 0) * s1))
    fista_m = register_op(
        "FISTA_M",
        Spec(body=minn(Src0 + Src1 * C0, Src1),
             reference=lambda in0, in1, s0, s1, imm2: np.minimum(in0 + in1 * s0, in1)))
    return fista_v, fista_z, fista_m


def _build(trace=False):
    key = (N_ITERS_RUN[0], LOOP_MODE[0], UNROLL[0], PASSES[0], ROUND_MODE[0],
           VARIANT[0], ZPP[0])
    if key in _cache:
        return _cache[key]
    import concourse.bacc as bacc
    import concourse.tile as tile
    from concourse import mybir
    import concourse.bass as bass

    FISTA_V, FISTA_Z, FISTA_M = _register_ops()
    step, k1, k2, s, alpha, wts_np, betas_np = _host_consts()
    f32 = mybir.dt.float32
    f32r = mybir.dt.float32r

    NBETA = max(N_ITERS, N_ITERS_RUN[0])
    nc = bacc.Bacc("TRN2", target_bir_lowering=False, debug=False)
    DATA = nc.dram_tensor("data", [NIMG_PER_CORE, N, N], f32, kind="ExternalInput")
    WTS = nc.dram_tensor("wts", [128, 6 * 128], f32, kind="ExternalInput")
    BET = nc.dram_tensor("betas", [128, 2 * NBETA], f32, kind="ExternalInput")
    OUT = nc.dram_tensor("out", [NIMG_PER_CORE, N, N], f32, kind="ExternalOutput")

    with tile.TileContext(nc) as tc:
        with (
            tc.tile_pool(name="state", bufs=1) as pool,
            tc.tile_pool(name="psum", bufs=2, space="PSUM") as psp,
            tc.tile_pool(name="scratch", bufs=(1 if ZPP[0] else 2)) as pool2,
        ):
            y_t = pool.tile([128, TOT], f32, tag="y")
            z_a = pool.tile([128, TOT], f32, tag="za")
            z_b = z_a
            zr_a = pool.tile([128, TOT], f32r, tag="zra")
            if ZPP[0]:
                zr_b = pool.tile([128, TOT], f32r, tag="zrb")
            else:
                zr_b = zr_a
            m_a = pool.tile([128, TOT], f32, tag="ma")
            m_b = pool.tile([128, TOT], f32, tag="mb")
            w_t = pool.tile([128, 6 * 128], f32, tag="w")
            wr_t = pool.tile([128, 5 * 128], f32r, tag="wr")
            bet_t = pool.tile([128, 2 * NBETA], f32, tag="bet")

            # ---- load weights/betas
            nc.sync.dma_start(w_t[:], WTS[:])
            nc.sync.dma_start(bet_t[:], BET[:])
            nc.vector.tensor_copy(wr_t[:], w_t[:, :5 * 128])
            ident = w_t[:, 5 * 128:6 * 128]
            WBLK = {0: 0, 1: 1, 2: 1, 3: 2}   # chunk -> diagonal lhsT slice

            def wslice(idx):
                return wr_t[:, idx * 128:(idx + 1) * 128]

            # ---- load input + transpose on PE:
            # y[p, c*FD + i*N + (128t+r')] = data[i, 128t+r', 128c+p]
            for i in range(NIMG_PER_CORE):
                for t_ in range(4):
                    sbase = ((i * 4 + t_) % 4) * N
                    nc.sync.dma_start(m_b[:, sbase:sbase + N],
                                      DATA[i, 128 * t_:128 * (t_ + 1), :])
                    psT = psp.tile([128, 512], f32, tag="ps")
                    for c in range(NCHUNK):
                        nc.tensor.transpose(psT[:, 128 * c:128 * (c + 1)],
                                            m_b[:, sbase + 128 * c:sbase + 128 * (c + 1)], ident)
                    dstv = y_t[:].rearrange("p (c n) -> p c n", c=NCHUNK)[
                        :, :, i * N + 128 * t_: i * N + 128 * (t_ + 1)]
                    nc.vector.tensor_copy(
                        dstv, psT[:, :N].rearrange("p (c n) -> p c n", c=NCHUNK))

            def round_to(dst, src):
                if ROUND_MODE[0] == "actcopy":
                    nc.scalar.activation(dst, src, mybir.ActivationFunctionType.Copy)
                elif ROUND_MODE[0] == "dvecopy":
                    nc.vector.tensor_copy(dst, src)
                else:
                    nc.gpsimd.tensor_copy(dst, src)

            def init_state():
                nc.vector.tensor_copy(z_a[:], y_t[:])
                nc.scalar.copy(m_a[:], y_t[:])
                nc.gpsimd.tensor_copy(zr_a[:], y_t[:])

            def iteration(m_in, m_out, zr_in, zr_out, i_b1, i_b0):
                z_in = z_out = z_a
                for c in range(NCHUNK):
                    cs = slice(c * FD, (c + 1) * FD)
                    ps = psp.tile([128, FD], mybir.dt.float32, tag="ps")
                    preload = VARIANT[0] == "d1"
                    if preload:
                        # PSUM preload: alpha * y
                        nc.scalar.activation(ps[:], y_t[:, cs],
                                             mybir.ActivationFunctionType.Copy,
                                             scale=float(alpha))
                    mlist = [(WBLK[c], c)]
                    if c + 1 < NCHUNK:
                        mlist.append((3, c + 1))
                    if c - 1 >= 0:
                        mlist.append((4, c - 1))
                    rhs_of = lambda cj, lo, hi: zr_in[:, cj * FD + lo: cj * FD + hi]
                    for p in range(FD // 512):
                        for bi, (widx, cj) in enumerate(mlist):
                            nc.tensor.matmul(
                                ps[:, p * 512:(p + 1) * 512], wslice(widx),
                                rhs_of(cj, p * 512, (p + 1) * 512),
                                start=(not preload and bi == 0),
                                stop=(bi == len(mlist) - 1),
                                skip_group_check=True,
                            )
                    if VARIANT[0] == "d1":
                        # u = (P - z*(k1/k2)) * (-k2)  [= k1 z - k2 A z + s y]
                        # written in-place into m_out, then clamped there
                        nc.vector._custom_dve(FISTA_V, out=m_out[:, cs], in0=ps[:],
                                              in1=z_in[:, cs], s0=k1 / k2, s1=-k2)
                        # m = min(u, y)  (Pool, in-place)
                        nc.gpsimd.tensor_tensor(m_out[:, cs], m_out[:, cs], y_t[:, cs],
                                                op=mybir.AluOpType.min)
                    else:
                        # u = (P - z*(k1/k2)) * (-k2)  [= k1 z - k2 A z]
                        # written in-place into m_out, then min'd there
                        nc.vector._custom_dve(FISTA_V, out=m_out[:, cs], in0=ps[:],
                                              in1=z_in[:, cs], s0=k1 / k2, s1=-k2)
                        # m = min(u + s*y, y)  (DVE custom, in-place)
                        nc.vector._custom_dve(FISTA_M, out=m_out[:, cs], in0=m_out[:, cs],
                                              in1=y_t[:, cs], s0=s)
                    # z' = b1*relu(m) - b0*relu(m_prev)
                    nc.vector._custom_dve(FISTA_Z, out=z_out[:, cs], in0=m_out[:, cs],
                                          in1=m_in[:, cs], s0=i_b1, s1=i_b0)
                    # zr = f32r(z') for the next iteration's matmuls
                    round_to(zr_out[:, cs], z_out[:, cs])

            def run_pass():
                nit = N_ITERS_RUN[0]
                if LOOP_MODE[0] == "unrolled":
                    for k in range(0, nit, 2):
                        iteration(m_a, m_b, zr_a, zr_b, bet_t[:, k:k + 1],
                                  bet_t[:, NBETA + k:NBETA + k + 1])
                        iteration(m_b, m_a, zr_b, zr_a, bet_t[:, k + 1:k + 2],
                                  bet_t[:, NBETA + k + 1:NBETA + k + 2])
                else:
                    U = UNROLL[0]
                    assert U % 2 == 0 and nit % U == 0
                    def body(i):
                        for j in range(0, U, 2):
                            iteration(m_a, m_b, zr_a, zr_b,
                                      bet_t[:, bass.ds(i + j, 1)],
                                      bet_t[:, bass.ds(i + j + NBETA, 1)])
                            iteration(m_b, m_a, zr_b, zr_a,
                                      bet_t[:, bass.ds(i + j + 1, 1)],
                                      bet_t[:, bass.ds(i + j + 1 + NBETA, 1)])
                    with tc.For_i(0, nit, U) as i:
                        body(i)

            init_state()
            run_pass()
            for _extra in range(PASSES[0] - 1):
                # next pass: y <- relu(m_a) (x_100), reinit, run again
                nc.scalar.activation(y_t[:], m_a[:], mybir.ActivationFunctionType.Relu)
                init_state()
                run_pass()
            # final x = relu(m_a) -> z_a as staging
            nc.scalar.activation(z_a[:], m_a[:], mybir.ActivationFunctionType.Relu)

            # store: transpose back on PE then contiguous DMA
            for i in range(NIMG_PER_CORE):
                for t_ in range(4):
                    psT = psp.tile([128, 512], f32, tag="ps")
                    for c in range(NCHUNK):
                        nc.tensor.transpose(
                            psT[:, 128 * c:128 * (c + 1)],
                            z_a[:, c * FD + i * N + 128 * t_: c * FD + i * N + 128 * (t_ + 1)],
                            ident)
                    S = m_b[:, ((i * 4 + t_) % 4) * N:(((i * 4 + t_) % 4) + 1) * N]
                    nc.vector.tensor_copy(S, psT[:, :N])
                    nc.sync.dma_start(OUT[i, 128 * t_:128 * (t_ + 1), :], S)

    nc.finalize()
    _cache[key] = nc
    return nc


def kernel(data: np.ndarray) -> np.ndarray:
    from concourse import bass_utils

    data = np.ascontiguousarray(data, np.float32)
    B = data.shape[0]
    nc = _build()
    _, _, _, _, _, wts_np, betas_np = _host_consts()
    in_maps = []
    for c in range(NCORES):
        in_maps.append({
            "data": np.ascontiguousarray(data[c * NIMG_PER_CORE:(c + 1) * NIMG_PER_CORE]),
            "wts": wts_np,
            "betas": betas_np,
        })
    res = bass_utils.run_bass_kernel_spmd(nc, in_maps, core_ids=list(range(NCORES)))
    out = np.concatenate([res.results[c]["out"] for c in range(NCORES)], axis=0)
    return out.reshape(B, N, N, 1)


if __name__ == "__main__":
    rng = np.random.default_rng(0)
    d = rng.random((32, N, N), dtype=np.float32)
    o = kernel(d)
    print("kernel ran, out shape", o.shape, "mean", o.mean())


# revision 4
# speedup vs baseline: 4.7676x; 1.2275x over previous
"""TRN2 Bass kernel for nn_Denoise: 2x100-iteration FISTA row denoise.

Scheme (per core, data-parallel over batch, 4 images/core):
  Layout: transposed per image -> tiles [128 elem-partitions, 8192] where
  column = chunk*2048 + img*512 + row  (4 chunks of 128 row-elements).
  Per iteration, per chunk (2048 cols):
    PE:           P  = A @ f32r(z)   (A = D^T D; 10 psum-accumulated matmuls)
    DVE custom V: u  = (P - z*(k1/k2)) * (-k2)  [= k1 z - k2 A z], in-place in m
    DVE custom M: m  = min(u + s*y, y)          (in-place)
    DVE custom Z: z' = b1*relu(m) - b0*relu(m_prev)
    ACT copy:     zr = f32r(z')      (mandatory rounding for the next matmul,
                                      off the DVE/PE critical engines)
  The fp32 k1*z path keeps full precision; f32r rounding enters only through
  A, which annihilates the smooth modes the FISTA map would amplify.

Perf notes (HW-measured via constant-NEFF wall contrast, no profiler here):
  - Dynamic For_i trips are expensive; the body unrolls 20 iterations
    (1140 instrs, the largest even divisor of 100 under the ~2k-instruction
    replay cliff; U=50 body regresses).
  - Register-offset APs in the body are the dominant trip cost: hoisting the
    trip's 40 beta scalars into a static staging tile (2 ACT copies with
    ds() register APs, all Z-ops use static offsets) cut per-iteration time
    from 35.7 us to 29.95 us (engine floor ~28 us, DVE-bound: 3 custom ops
    x 4 chunks at 1x rate).
  - zr and m are double-buffered so cross-iteration WAR hazards do not
    serialize the 4-chunk pipeline; V and M write in-place.
  - Measured: 29.95 us/iter -> 5.99 ms for 200 iterations (baseline 27.7 ms).
"""
import numpy as np

LAM = 10.0
N_ITERS = 100
N = 512
NIMG_PER_CORE = 4
NCORES = 8
FD = NIMG_PER_CORE * N          # 2048 columns per chunk
NCHUNK = 4                      # 512 = 4 * 128 row-elements
TOT = NCHUNK * FD               # 8192

_cache = {}

# knobs (module-level so experiments can flip them)
N_ITERS_RUN = [N_ITERS]
PASSES = [2]
LOOP_MODE = ["dynamic"]         # "dynamic" | "unrolled"
UNROLL = [20]                   # iterations per loop trip in dynamic mode
ROUND_MODE = ["actcopy"]        # "actcopy" | "dvecopy" | "poolcopy"
VARIANT = ["va"]                # "va": 3 DVE customs, no preload, no pool
                                # "d1": ACT preload + pool min + 2 DVE customs
ZPP = [True]                   # ping-pong z tiles
OUTER_REPS = [1]                # timing-only: repeat the whole pass loop
WIDE = [False]                  # M/Z/round at 4096 (2-chunk groups)
BETA_HOIST = [True]            # hoist betas per trip -> static APs in body


def _host_consts(nbeta=None):
    if nbeta is None:
        nbeta = max(N_ITERS, N_ITERS_RUN[0])
    step = np.float32(1.0 / (2.0 * (1.0 + 16.0 * LAM)))
    k1 = float(1.0 - 2.0 * float(step))
    k2 = float(2.0 * LAM * float(step))
    s = float(2.0 * float(step))
    alpha = -s / k2  # = -1/LAM
    t = np.float32(1.0)
    b0 = np.zeros(nbeta, np.float32)
    b1 = np.zeros(nbeta, np.float32)
    for k in range(nbeta):
        tn = np.float32(0.5) * (np.float32(1.0) + np.sqrt(np.float32(1.0) + np.float32(4.0) * t * t))
        beta = (t - np.float32(1.0)) / tn
        b0[k] = beta
        b1[k] = np.float32(1.0) + beta
        t = tn
    D = np.zeros((N - 2, N), np.float64)
    idx = np.arange(N - 2)
    D[idx, idx] = 1.0
    D[idx, idx + 1] = -2.0
    D[idx, idx + 2] = 1.0
    A = D.T @ D
    A00 = A[0:128, 0:128]
    AII = A[128:256, 128:256]
    A33 = A[384:512, 384:512]
    BU = A[128:256, 0:128]      # cj = c+1 feeding c
    BL = A[0:128, 128:256]      # cj = c-1 feeding c
    wts = np.concatenate([A00, AII, A33, BU, BL, np.eye(128)], axis=1).astype(np.float32)  # [128, 768]
    betas = np.concatenate([b1, b0])[None, :].repeat(128, axis=0).astype(np.float32)  # [128, 200]
    return step, k1, k2, s, alpha, wts, betas


def _register_ops():
    from concourse import dve_ops
    from concourse.dve_spec import Spec, Src0, Src1, C0, C1, lower
    from concourse.dve_spec import _has_src1 as has_src1
    from concourse.dve_spec import relu as drelu
    from concourse.dve_uop import DveOpSpec

    def register_op(name, spec):
        for o in dve_ops.OPS:
            if o.name == name:
                return o
        op = dve_ops.DveOp(name, spec, False, uops_sha={})
        dve_ops.OPS.append(op)
        row = dve_ops._CUSTOM_DVE_ROW_BASE + len(dve_ops.OPS) - 1
        assert row < 0x20
        dve_ops._SUB_OPCODE_FOR_NAME[name] = row
        dve_ops.CUSTOM_DVE_SPECS[name] = spec
        sha = {}
        for ver in ("v3", "v4"):
            sp = DveOpSpec(name=name, opcode=row, uops=lower(spec, ver=ver),
                           rd1_en=has_src1(spec))
            sha[ver] = sp.sha(ver)
        op2 = dve_ops.DveOp(name, spec, False, uops_sha=sha)
        dve_ops.OPS[-1] = op2
        return op2

    from concourse.dve_spec import minn
    fista_v = register_op(
        "FISTA_V",
        Spec(body=(Src0 - Src1 * C0) * C1,
             reference=lambda in0, in1, s0, s1, imm2: (in0 - in1 * s0) * s1))
    fista_z = register_op(
        "FISTA_Z",
        Spec(body=drelu(Src0) * C0 - drelu(Src1) * C1,
             reference=lambda in0, in1, s0, s1, imm2:
             np.maximum(in0, 0) * s0 - np.maximum(in1, 0) * s1))
    fista_m = register_op(
        "FISTA_M",
        Spec(body=minn(Src0 + Src1 * C0, Src1),
             reference=lambda in0, in1, s0, s1, imm2: np.minimum(in0 + in1 * s0, in1)))
    return fista_v, fista_z, fista_m


def _build(trace=False):
    key = (N_ITERS_RUN[0], LOOP_MODE[0], UNROLL[0], PASSES[0], ROUND_MODE[0],
           VARIANT[0], ZPP[0], OUTER_REPS[0], WIDE[0], BETA_HOIST[0])
    if key in _cache:
        return _cache[key]
    import concourse.bacc as bacc
    import concourse.tile as tile
    from concourse import mybir
    import concourse.bass as bass

    FISTA_V, FISTA_Z, FISTA_M = _register_ops()
    step, k1, k2, s, alpha, wts_np, betas_np = _host_consts()
    f32 = mybir.dt.float32
    f32r = mybir.dt.float32r

    NBETA = max(N_ITERS, N_ITERS_RUN[0])
    nc = bacc.Bacc("TRN2", target_bir_lowering=False, debug=False)
    DATA = nc.dram_tensor("data", [NIMG_PER_CORE, N, N], f32, kind="ExternalInput")
    WTS = nc.dram_tensor("wts", [128, 6 * 128], f32, kind="ExternalInput")
    BET = nc.dram_tensor("betas", [128, 2 * NBETA], f32, kind="ExternalInput")
    OUT = nc.dram_tensor("out", [NIMG_PER_CORE, N, N], f32, kind="ExternalOutput")

    with tile.TileContext(nc) as tc:
        with (
            tc.tile_pool(name="state", bufs=1) as pool,
            tc.tile_pool(name="psum", bufs=2, space="PSUM") as psp,
            tc.tile_pool(name="scratch", bufs=(1 if ZPP[0] else 2)) as pool2,
        ):
            y_t = pool.tile([128, TOT], f32, tag="y")
            z_a = pool.tile([128, TOT], f32, tag="za")
            z_b = z_a
            zr_a = pool.tile([128, TOT], f32r, tag="zra")
            if ZPP[0]:
                zr_b = pool.tile([128, TOT], f32r, tag="zrb")
            else:
                zr_b = zr_a
            m_a = pool.tile([128, TOT], f32, tag="ma")
            m_b = pool.tile([128, TOT], f32, tag="mb")
            w_t = pool.tile([128, 6 * 128], f32, tag="w")
            wr_t = pool.tile([128, 5 * 128], f32r, tag="wr")
            bet_t = pool.tile([128, 2 * NBETA], f32, tag="bet")
            bst_t = pool.tile([128, 2 * max(UNROLL[0], 2)], f32, tag="bst")

            # ---- load weights/betas
            nc.sync.dma_start(w_t[:], WTS[:])
            nc.sync.dma_start(bet_t[:], BET[:])
            nc.vector.tensor_copy(wr_t[:], w_t[:, :5 * 128])
            ident = w_t[:, 5 * 128:6 * 128]
            WBLK = {0: 0, 1: 1, 2: 1, 3: 2}   # chunk -> diagonal lhsT slice

            def wslice(idx):
                return wr_t[:, idx * 128:(idx + 1) * 128]

            # ---- load input + transpose on PE:
            # y[p, c*FD + i*N + (128t+r')] = data[i, 128t+r', 128c+p]
            for i in range(NIMG_PER_CORE):
                for t_ in range(4):
                    sbase = ((i * 4 + t_) % 4) * N
                    nc.sync.dma_start(m_b[:, sbase:sbase + N],
                                      DATA[i, 128 * t_:128 * (t_ + 1), :])
                    psT = psp.tile([128, 512], f32, tag="ps")
                    for c in range(NCHUNK):
                        nc.tensor.transpose(psT[:, 128 * c:128 * (c + 1)],
                                            m_b[:, sbase + 128 * c:sbase + 128 * (c + 1)], ident)
                    dstv = y_t[:].rearrange("p (c n) -> p c n", c=NCHUNK)[
                        :, :, i * N + 128 * t_: i * N + 128 * (t_ + 1)]
                    nc.vector.tensor_copy(
                        dstv, psT[:, :N].rearrange("p (c n) -> p c n", c=NCHUNK))

            def round_to(dst, src):
                if ROUND_MODE[0] == "actcopy":
                    nc.scalar.activation(dst, src, mybir.ActivationFunctionType.Copy)
                elif ROUND_MODE[0] == "dvecopy":
                    nc.vector.tensor_copy(dst, src)
                else:
                    nc.gpsimd.tensor_copy(dst, src)

            def init_state():
                nc.vector.tensor_copy(z_a[:], y_t[:])
                nc.scalar.copy(m_a[:], y_t[:])
                nc.gpsimd.tensor_copy(zr_a[:], y_t[:])

            def chunk_pe_v(c, m_out, z_in, zr_in):
                """PE matmuls + in-place V for one 2048-col chunk."""
                cs = slice(c * FD, (c + 1) * FD)
                ps = psp.tile([128, FD], mybir.dt.float32, tag="ps")
                mlist = [(WBLK[c], c)]
                if c + 1 < NCHUNK:
                    mlist.append((3, c + 1))
                if c - 1 >= 0:
                    mlist.append((4, c - 1))
                rhs_of = lambda cj, lo, hi: zr_in[:, cj * FD + lo: cj * FD + hi]
                for p in range(FD // 512):
                    for bi, (widx, cj) in enumerate(mlist):
                        nc.tensor.matmul(
                            ps[:, p * 512:(p + 1) * 512], wslice(widx),
                            rhs_of(cj, p * 512, (p + 1) * 512),
                            start=(bi == 0),
                            stop=(bi == len(mlist) - 1),
                            skip_group_check=True,
                        )
                # u = (P - z*(k1/k2)) * (-k2)  [= k1 z - k2 A z], in-place in m
                nc.vector._custom_dve(FISTA_V, out=m_out[:, cs], in0=ps[:],
                                      in1=z_in[:, cs], s0=k1 / k2, s1=-k2)

            def iteration(m_in, m_out, zr_in, zr_out, i_b1, i_b0):
                z_in = z_out = z_a
                if WIDE[0]:
                    # groups of 2 chunks: per-chunk PE+V, then 4096-wide M/Z/round
                    for g in range(NCHUNK // 2):
                        gs = slice(2 * g * FD, 2 * (g + 1) * FD)
                        for c in (2 * g, 2 * g + 1):
                            chunk_pe_v(c, m_out, z_in, zr_in)
                        nc.vector._custom_dve(FISTA_M, out=m_out[:, gs], in0=m_out[:, gs],
                                              in1=y_t[:, gs], s0=s)
                        nc.vector._custom_dve(FISTA_Z, out=z_out[:, gs], in0=m_out[:, gs],
                                              in1=m_in[:, gs], s0=i_b1, s1=i_b0)
                        round_to(zr_out[:, gs], z_out[:, gs])
                    return
                for c in range(NCHUNK):
                    cs = slice(c * FD, (c + 1) * FD)
                    chunk_pe_v(c, m_out, z_in, zr_in)
                    # m = min(u + s*y, y)  (DVE custom, in-place)
                    nc.vector._custom_dve(FISTA_M, out=m_out[:, cs], in0=m_out[:, cs],
                                          in1=y_t[:, cs], s0=s)
                    # z' = b1*relu(m) - b0*relu(m_prev)
                    nc.vector._custom_dve(FISTA_Z, out=z_out[:, cs], in0=m_out[:, cs],
                                          in1=m_in[:, cs], s0=i_b1, s1=i_b0)
                    # zr = f32r(z') for the next iteration's matmuls
                    round_to(zr_out[:, cs], z_out[:, cs])

            def run_pass():
                nit = N_ITERS_RUN[0]
                if LOOP_MODE[0] == "unrolled":
                    for k in range(0, nit, 2):
                        iteration(m_a, m_b, zr_a, zr_b, bet_t[:, k:k + 1],
                                  bet_t[:, NBETA + k:NBETA + k + 1])
                        iteration(m_b, m_a, zr_b, zr_a, bet_t[:, k + 1:k + 2],
                                  bet_t[:, NBETA + k + 1:NBETA + k + 2])
                else:
                    U = UNROLL[0]
                    assert U % 2 == 0 and nit % U == 0
                    def body(i):
                        if BETA_HOIST[0]:
                            # hoist this trip's betas to static offsets (ACT),
                            # so the 2U Z-ops in the body carry no register APs
                            nc.scalar.activation(bst_t[:, 0:U], bet_t[:, bass.ds(i, U)],
                                                 mybir.ActivationFunctionType.Copy)
                            nc.scalar.activation(bst_t[:, U:2 * U],
                                                 bet_t[:, bass.ds(i + NBETA, U)],
                                                 mybir.ActivationFunctionType.Copy)
                            b1_of = lambda j: bst_t[:, j:j + 1]
                            b0_of = lambda j: bst_t[:, U + j:U + j + 1]
                        else:
                            b1_of = lambda j: bet_t[:, bass.ds(i + j, 1)]
                            b0_of = lambda j: bet_t[:, bass.ds(i + j + NBETA, 1)]
                        for j in range(0, U, 2):
                            iteration(m_a, m_b, zr_a, zr_b, b1_of(j), b0_of(j))
                            iteration(m_b, m_a, zr_b, zr_a, b1_of(j + 1), b0_of(j + 1))
                    if OUTER_REPS[0] > 1:
                        with tc.For_i(0, OUTER_REPS[0], 1) as _r:
                            with tc.For_i(0, nit, U) as i:
                                body(i)
                    else:
                        with tc.For_i(0, nit, U) as i:
                            body(i)

            init_state()
            run_pass()
            for _extra in range(PASSES[0] - 1):
                # next pass: y <- relu(m_a) (x_100), reinit, run again
                nc.scalar.activation(y_t[:], m_a[:], mybir.ActivationFunctionType.Relu)
                init_state()
                run_pass()
            # final x = relu(m_a) -> z_a as staging
            nc.scalar.activation(z_a[:], m_a[:], mybir.ActivationFunctionType.Relu)

            # store: transpose back on PE then contiguous DMA
            for i in range(NIMG_PER_CORE):
                for t_ in range(4):
                    psT = psp.tile([128, 512], f32, tag="ps")
                    for c in range(NCHUNK):
                        nc.tensor.transpose(
                            psT[:, 128 * c:128 * (c + 1)],
                            z_a[:, c * FD + i * N + 128 * t_: c * FD + i * N + 128 * (t_ + 1)],
                            ident)
                    S = m_b[:, ((i * 4 + t_) % 4) * N:(((i * 4 + t_) % 4) + 1) * N]
                    nc.vector.tensor_copy(S, psT[:, :N])
                    nc.sync.dma_start(OUT[i, 128 * t_:128 * (t_ + 1), :], S)

    nc.finalize()
    _cache[key] = nc
    return nc


def kernel(data: np.ndarray) -> np.ndarray:
    from concourse import bass_utils

    data = np.ascontiguousarray(data, np.float32)
    B = data.shape[0]
    nc = _build()
    _, _, _, _, _, wts_np, betas_np = _host_consts()
    in_maps = []
    for c in range(NCORES):
        in_maps.append({
            "data": np.ascontiguousarray(data[c * NIMG_PER_CORE:(c + 1) * NIMG_PER_CORE]),
            "wts": wts_np,
            "betas": betas_np,
        })
    res = bass_utils.run_bass_kernel_spmd(nc, in_maps, core_ids=list(range(NCORES)))
    out = np.concatenate([res.results[c]["out"] for c in range(NCORES)], axis=0)
    return out.reshape(B, N, N, 1)


if __name__ == "__main__":
    rng = np.random.default_rng(0)
    d = rng.random((32, N, N), dtype=np.float32)
    o = kernel(d)
    print("kernel ran, out shape", o.shape, "mean", o.mean())
